# revision 2
# baseline (speedup 1.0000x reference)
"""CrossNetMix (moe_routing) Trainium2 Bass kernel.

Math (per layer i, with softmax gates g summing to 1 over E):
    x_{l+1} = x_l + x0 * (sum_e g_e * U_e @ tanh(C_e @ tanh(V_e^T x_l)) + bias_i)

Everything runs in transposed space y = x^T [D, B] so all matmuls contract
over the partition axis with no activation transposes:
  - S0 gating:  glog[4,B]  = G^T-chunks (lhsT) x y          (8 K-chunks, PSUM acc)
  - softmax:    eg = exp(glog); Z4 = ones44 x eg; rZ = 1/Z; gn4 = eg*rZ
  - broadcast:  gbc[256,B] = Sel x gn4                      (2 matmuls)
  - S1 V-stage: v[256,B]   = packed-V-pairs (lhsT) x y      (2x8 matmuls, PSUM acc)
  - S2 C-stage: w[256,B]   = blockdiag-C^T x tanh(v)        (2 matmuls)
  - wg = tanh(w) * gbc
  - S3 U-stage: acc[1024,B] = packed-U x wg                 (8 M x 2 K matmuls)
  - final:      y_new = y + y0 * (acc + bias)   (stt on DVE + add on DVE/GPSIMD)

Matmul operands use float32r (full-rate PE, ~1e-4 matmul accuracy).
Batch B=16384 is sharded over 8 cores (2048 each), processed in 4 chunks
of 512 columns (= max fp32 PSUM moving dim).
"""

import numpy as np

B, D, R, E, L = 16384, 1024, 64, 4, 3
NCORES = 8
BC = B // NCORES            # columns per core
BT = 512                    # columns per chunk (= PSUM bank fp32 capacity)
NCHUNK = BC // BT
KC = D // 128               # K-chunks over D
NM = D // 128               # M-chunks over D

# which final-add m-chunks go to GPSIMD (rest on DVE)
ADD_ON_GPSIMD = (0, 1, 2, 4, 5, 6)

_CACHE = {}


def _build(bias_nonzero: bool):
    import concourse.mybir as mybir
    import concourse.bacc as bacc
    import concourse.tile as tile

    f32 = mybir.dt.float32
    f32r = mybir.dt.float32r
    ALU = mybir.AluOpType
    ACTF = mybir.ActivationFunctionType

    nc = bacc.Bacc("TRN2", target_bir_lowering=False, debug=False,
                   num_devices=NCORES)

    xT = nc.dram_tensor("xT", [KC, 128, BC], f32r, kind="ExternalInput")
    GT = nc.dram_tensor("GT", [KC, 128, E], f32r, kind="ExternalInput")
    VP = nc.dram_tensor("VP", [L, KC, 128, 2, 128], f32r, kind="ExternalInput")
    CB = nc.dram_tensor("CB", [L, 2, 128, 128], f32r, kind="ExternalInput")
    UP = nc.dram_tensor("UP", [L, 2, 128, NM, 128], f32r, kind="ExternalInput")
    SEL = nc.dram_tensor("SEL", [E, 2, 128], f32r, kind="ExternalInput")
    ONES = nc.dram_tensor("ONES", [E, E], f32r, kind="ExternalInput")
    BIA = nc.dram_tensor("BIA", [128, L * NM], f32, kind="ExternalInput")
    outT = nc.dram_tensor("outT", [KC, 128, BC], f32r, kind="ExternalOutput")

    with tile.TileContext(nc) as tc:
        with (
            tc.tile_pool(name="wts", bufs=1) as wts,
            tc.tile_pool(name="y0p", bufs=2) as y0p,
            tc.tile_pool(name="yp", bufs=3) as yp,
            tc.tile_pool(name="tp", bufs=3) as tp,
            tc.tile_pool(name="twp", bufs=3) as twp,
            tc.tile_pool(name="wgp", bufs=3) as wgp,
            tc.tile_pool(name="gp", bufs=3) as gp,
            tc.tile_pool(name="tmpp", bufs=5) as tmpp,
            tc.tile_pool(name="ps_g", bufs=1, space="PSUM") as ps_g,
            tc.tile_pool(name="ps_gbc", bufs=2, space="PSUM") as ps_gbc,
            tc.tile_pool(name="ps_vw", bufs=3, space="PSUM") as ps_vw,
            tc.tile_pool(name="ps_acc", bufs=2, space="PSUM") as ps_acc,
        ):
            # ---- load weights (resident) ----
            gt_sb = wts.tile([128, KC, E], f32r, tag="gt")
            nc.sync.dma_start(out=gt_sb[:], in_=GT.rearrange("kc p e -> p kc e"))
            sel_sb = wts.tile([E, 2, 128], f32r, tag="sel")
            nc.sync.dma_start(out=sel_sb[:], in_=SEL[:, :, :])
            ones_sb = wts.tile([E, E], f32r, tag="ones")
            nc.sync.dma_start(out=ones_sb[:], in_=ONES[:, :])
            bia_sb = wts.tile([128, L * NM], f32, tag="bia")
            nc.sync.dma_start(out=bia_sb[:], in_=BIA[:, :])
            vp_sb, cb_sb, up_sb = [], [], []
            for i in range(L):
                v = wts.tile([128, KC, 2, 128], f32r, tag=f"vp{i}")
                nc.sync.dma_start(out=v[:], in_=VP[i].rearrange("kc p pr m -> p kc pr m"))
                vp_sb.append(v)
                c = wts.tile([128, 2, 128], f32r, tag=f"cb{i}")
                nc.sync.dma_start(out=c[:], in_=CB[i].rearrange("k2 p m -> p k2 m"))
                cb_sb.append(c)
                u = wts.tile([128, 2, NM, 128], f32r, tag=f"up{i}")
                nc.sync.dma_start(out=u[:], in_=UP[i].rearrange("k2 p mc m -> p k2 mc m"))
                up_sb.append(u)

            # ---- main loop over chunks ----
            for cidx in range(NCHUNK):
                c0 = cidx * BT
                y0 = y0p.tile([128, KC, BT], f32r, tag="y0")
                nc.sync.dma_start(
                    out=y0[:], in_=xT[:, :, c0:c0 + BT].rearrange("kc p b -> p kc b"))
                y_in = y0
                for i in range(L):
                    # --- expert V-stage (16 matmuls) ---
                    v_ps = [ps_vw.tile([128, BT], f32, tag="vw", name=f"v{pr_}") for pr_ in range(2)]
                    for pr in range(2):
                        for k in range(KC):
                            nc.tensor.matmul(
                                v_ps[pr][:], vp_sb[i][:, k, pr, :], y_in[:, k, :],
                                start=(k == 0), stop=(k == KC - 1))
                    # --- gating logits (8 matmuls) ---
                    glog = ps_g.tile([E, BT], f32, tag="g")
                    for k in range(KC):
                        nc.tensor.matmul(glog[:], gt_sb[:, k, :], y_in[:, k, :],
                                         start=(k == 0), stop=(k == KC - 1))
                    # tanh(v) while glog accumulates
                    t_sb = [tp.tile([128, BT], f32r, tag="t", name=f"t{pr_}") for pr_ in range(2)]
                    for pr in range(2):
                        nc.scalar.activation(t_sb[pr][:], v_ps[pr][:], ACTF.Tanh)
                    # softmax pieces
                    eg = gp.tile([E, BT], f32r, tag="eg")
                    nc.scalar.activation(eg[:], glog[:], ACTF.Exp)
                    z4 = ps_g.tile([E, BT], f32, tag="g")
                    nc.tensor.matmul(z4[:], ones_sb[:], eg[:], start=True, stop=True)
                    rz4 = gp.tile([E, BT], f32, tag="rz")
                    nc.vector.reciprocal_approx_fast(out=rz4[:], in_=z4[:])
                    gn4 = gp.tile([E, BT], f32r, tag="gn")
                    nc.vector.tensor_mul(out=gn4[:], in0=eg[:], in1=rz4[:])
                    # --- C-stage ---
                    w_ps = [ps_vw.tile([128, BT], f32, tag="vw", name=f"w{pr_}") for pr_ in range(2)]
                    for pr in range(2):
                        nc.tensor.matmul(w_ps[pr][:], cb_sb[i][:, pr, :], t_sb[pr][:],
                                         start=True, stop=True)
                    # gate broadcast [4,BT] -> [256,BT]
                    gbc_ps = [ps_gbc.tile([128, BT], f32, tag="gbc", name=f"gbc{pr_}") for pr_ in range(2)]
                    for pr in range(2):
                        nc.tensor.matmul(gbc_ps[pr][:], sel_sb[:, pr, :], gn4[:],
                                         start=True, stop=True)
                    # tanh(w), then gate-scale
                    wg_sb = []
                    for pr in range(2):
                        tw = twp.tile([128, BT], f32, tag="tw")
                        nc.scalar.activation(tw[:], w_ps[pr][:], ACTF.Tanh)
                        wg = wgp.tile([128, BT], f32r, tag="wg")
                        nc.vector.tensor_mul(out=wg[:], in0=tw[:], in1=gbc_ps[pr][:])
                        wg_sb.append(wg)
                    # --- U-stage + final update ---
                    y_out = yp.tile([128, KC, BT], f32r, tag="y")
                    for m in range(NM):
                        acc = ps_acc.tile([128, BT], f32, tag="acc")
                        nc.tensor.matmul(acc[:], up_sb[i][:, 0, m, :], wg_sb[0][:],
                                         start=True, stop=False)
                        nc.tensor.matmul(acc[:], up_sb[i][:, 1, m, :], wg_sb[1][:],
                                         start=False, stop=True)
                        tmp = tmpp.tile([128, BT], f32, tag="tmp")
                        if bias_nonzero:
                            nc.vector.scalar_tensor_tensor(
                                out=tmp[:], in0=acc[:],
                                scalar=bia_sb[:, i * NM + m: i * NM + m + 1],
                                in1=y0[:, m, :], op0=ALU.add, op1=ALU.mult)
                        else:
                            nc.vector.tensor_mul(out=tmp[:], in0=acc[:],
                                                 in1=y0[:, m, :])
                        eng = nc.gpsimd if m in ADD_ON_GPSIMD else nc.vector
                        eng.tensor_add(out=y_out[:, m, :], in0=y_in[:, m, :],
                                       in1=tmp[:])
                    y_in = y_out
                nc.sync.dma_start(
                    out=outT[:, :, c0:c0 + BT].rearrange("kc p b -> p kc b"),
                    in_=y_in[:])
    nc.compile()
    return nc


def _get_nc(bias_nonzero: bool):
    key = ("nc", bias_nonzero)
    if key not in _CACHE:
        _CACHE[key] = _build(bias_nonzero)
    return _CACHE[key]


def _prep_inputs(inputs, U, V, C, G, bias):
    """Host-side layout prep -> per-core in_maps."""
    f32 = np.float32
    xT = np.ascontiguousarray(inputs.T.astype(f32, copy=False))  # [D, B]
    GTh = np.ascontiguousarray(G.T).reshape(KC, 128, E).astype(f32, copy=False)
    VPh = np.ascontiguousarray(
        V.transpose(0, 2, 1, 3).reshape(L, D, E * R).reshape(L, KC, 128, 2, 128))
    CBh = np.zeros((L, 2, 128, 128), f32)
    for i in range(L):
        for pr in range(2):
            CBh[i, pr, :64, :64] = C[i, 2 * pr].T
            CBh[i, pr, 64:, 64:] = C[i, 2 * pr + 1].T
    UPh = np.ascontiguousarray(
        U.transpose(0, 1, 3, 2).reshape(L, E * R, D).reshape(L, 2, 128, NM, 128))
    SELh = np.zeros((E, 2 * 128), f32)
    for e in range(E):
        SELh[e, e * 64:(e + 1) * 64] = 1.0
    SELh = SELh.reshape(E, 2, 128)
    ONESh = np.ones((E, E), f32)
    BIAh = np.ascontiguousarray(
        bias.reshape(L, NM, 128).transpose(2, 0, 1).reshape(128, L * NM)
    ).astype(f32, copy=False)

    in_maps = []
    for c in range(NCORES):
        xTc = np.ascontiguousarray(
            xT[:, c * BC:(c + 1) * BC]).reshape(KC, 128, BC)
        in_maps.append({
            "xT": xTc, "GT": GTh, "VP": VPh, "CB": CBh, "UP": UPh,
            "SEL": SELh, "ONES": ONESh, "BIA": BIAh,
        })
    return in_maps


def kernel(inputs, U, V, C, G, bias):
    from concourse.bass_utils import run_bass_kernel_spmd

    inputs = np.asarray(inputs, dtype=np.float32)
    U = np.asarray(U, dtype=np.float32)
    V = np.asarray(V, dtype=np.float32)
    C = np.asarray(C, dtype=np.float32)
    G = np.asarray(G, dtype=np.float32)
    bias = np.asarray(bias, dtype=np.float32)

    bias_nonzero = bool(np.any(bias != 0.0))
    nc = _get_nc(bias_nonzero)
    in_maps = _prep_inputs(inputs, U, V, C, G, bias)
    res = run_bass_kernel_spmd(nc, in_maps, core_ids=list(range(NCORES)))
    out = np.empty((D, B), np.float32)
    for c in range(NCORES):
        out[:, c * BC:(c + 1) * BC] = res.results[c]["outT"].reshape(D, BC)
    return np.ascontiguousarray(out.T)
